# revision 9
# baseline (speedup 1.0000x reference)
"""GAT layer kernel for Trainium2, 8 NeuronCores, data-parallel over batch.

Per batch b (one core each):
    h   = x @ W;  a1 = x @ a[:D];  a2 = x @ a[D:]
    e   = leaky_relu(a1[i] + a2[j], 0.2)
    att = softmax over i of where(adj>0, e, -inf)
    out = elu(att-weighted h) @ han_w + han_b

Key reformulation (exact): with z = a1[i] + a2[j],
    exp(leakyrelu(z)) = exp(a2) * v1[i] * max(1, r[j] * w[i])
where v1 = exp(a1), w = exp(-0.8 a1), r = exp(-0.8 a2); the exp(a2)
factor cancels in the softmax column normalization.  Per j-row we need
pt[j,i] = gate[j,i] * v1[i]/8 * max(1, r[j]w[i]) and its row sum.
gate*v1/8 comes out of one PE matmul of the natural-layout adj tile
against diag(v1/8) (transpose + scale + gate in one pass, fp32 PSUM
quarters).  max(1, r[j]*w[i]) is one 4x-mode tensor_scalar on a bf16
broadcast w tile.  pt and the softmax row sum come from one
scalar_tensor_tensor with accum_out per quarter.  The row-sum division
folds into hs = h/rowsum; elu's "-1" folds into b_eff.  The i-half-0
part of h'^T = hs^T @ pt accumulates inside the attention loop; the
i-half-1 sweeps run in the tail interleaved with elu + out-projection.
"""

import math

import numpy as np

import concourse.bacc as bacc
import concourse.mybir as mybir
from concourse import masks
from concourse.tile import TileContext
from concourse.bass_utils import run_bass_kernel_spmd

P = 128
N = 2048
D = 256
NT = N // P          # 16 node tiles
DC = D // P          # 2 d chunks
NH = N // 2          # i-half size
NQ = N // 4          # i-quarter size
LN8 = math.log(8.0)

dt = mybir.dt
AF = mybir.ActivationFunctionType
OP = mybir.AluOpType

_CACHED_NC = None


def build_nc():
    nc = bacc.Bacc("TRN2", target_bir_lowering=False, debug=False)

    x_d = nc.dram_tensor("x16", [N, D], dt.float16, kind="ExternalInput")
    adj_d = nc.dram_tensor("adj16", [N, N], dt.float16, kind="ExternalInput")
    w_d = nc.dram_tensor("W16", [D, D], dt.float16, kind="ExternalInput")
    a_d = nc.dram_tensor("a_rs", [D, 2], dt.float16, kind="ExternalInput")
    han_d = nc.dram_tensor("han16", [D, D], dt.float16, kind="ExternalInput")
    beff_d = nc.dram_tensor("b_eff", [1, D], dt.float16, kind="ExternalInput")
    out_d = nc.dram_tensor("out", [N, D], dt.float32, kind="ExternalOutput")

    adj_r = adj_d.rearrange("(it p) j -> p it j", p=P)

    with TileContext(nc) as tc:
        with (
            tc.tile_pool(name="const", bufs=1) as cp,
            tc.tile_pool(name="big", bufs=1) as bp,
        ):
            ident16 = cp.tile([P, P], dt.float16)
            masks.make_identity(nc, ident16[:])
            ones_bf = cp.tile([1, P], dt.bfloat16)
            nc.vector.memset(ones_bf[:], 1.0)
            ones_f16 = cp.tile([1, P], dt.float16)
            nc.vector.memset(ones_f16[:], 1.0)
            nln8 = cp.tile([P, 1], dt.float32)
            nc.vector.memset(nln8[:], -LN8)

            w_bf = cp.tile([P, DC * D], dt.float16)
            nc.gpsimd.dma_start(w_bf[:].rearrange("p (c d) -> p c d", c=DC),
                                w_d.rearrange("(c p) d -> p c d", p=P))
            han_bf = cp.tile([P, DC * D], dt.float16)
            nc.gpsimd.dma_start(han_bf[:].rearrange("p (c d) -> p c d", c=DC),
                                han_d.rearrange("(c p) d -> p c d", p=P))
            a_f = cp.tile([P, DC * 2], dt.float16)
            nc.gpsimd.dma_start(a_f[:].rearrange("p (c t) -> p c t", c=DC),
                                a_d.rearrange("(c p) t -> p c t", p=P))
            beff_sb = cp.tile([1, D], dt.float16)
            nc.gpsimd.dma_start(beff_sb[:], beff_d[:])

            # persistent tensors
            xT = bp.tile([P, DC * N], dt.float16, tag="xT", name="xT")
            h_all = [bp.tile([P, D], dt.float16, tag=f"h{i}", name=f"h{i}")
                     for i in range(NT)]
            hs_all = [bp.tile([P, D], dt.float16, tag=f"hs{i}", name=f"hs{i}")
                      for i in range(NT)]
            pt_all = [bp.tile([P, N], dt.float16, tag=f"pt{i}", name=f"pt{i}")
                      for i in range(NT)]
            v1d = [bp.tile([P, P], dt.float16, tag=f"v1d{i}", name=f"v1d{i}")
                   for i in range(NT)]
            t_all = [bp.tile([P, N], dt.bfloat16, tag=f"t{i % 2}",
                             name=f"t{i % 2}") for i in range(2)]
            adj_sb = [bp.tile([P, NT * 2 * P], dt.float16, tag=f"adj{i}",
                              name=f"adj{i}") for i in range(2)]
            wbc = bp.tile([P, N], dt.bfloat16, tag="wbc", name="wbc")
            wrow = bp.tile([1, N], dt.bfloat16, tag="wrow", name="wrow")
            ac_sb = bp.tile([P, 2 * NT], dt.float32, tag="ac", name="ac")
            v1c = bp.tile([P, NT], dt.float32, tag="v1c", name="v1c")
            r_all = bp.tile([P, NT], dt.float32, tag="r", name="r")
            rsh = [bp.tile([P, 4], dt.float32, tag=f"rsh{i}", name=f"rsh{i}")
                   for i in range(NT)]
            rs_s = [bp.tile([P, 4], dt.float32, tag=f"rss{i}", name=f"rss{i}")
                    for i in range(NT)]                # Act identity scratch
            rs_p = [bp.tile([P, 2], dt.float32, tag=f"rsp{i}", name=f"rsp{i}")
                    for i in range(NT // 2)]
            rc_p = [bp.tile([P, 2], dt.float32, tag=f"rcp{i}", name=f"rcp{i}")
                    for i in range(NT // 2)]
            A_bf = [bp.tile([P, N], dt.float16, tag=f"A{c}", name=f"A{c}")
                    for c in range(DC)]

            def adj_load(pr):
                nc.sync.dma_start(
                    adj_sb[pr % 2][:].rearrange("p (it j) -> p it j", j=2 * P),
                    adj_r[:, :, pr * 2 * P:(pr + 1) * 2 * P])

            # ---- stage 1: x load/transpose, a-matmuls, row exps
            XG = 4
            with (
                tc.tile_pool(name="xload", bufs=4) as xl,
                tc.tile_pool(name="xps", bufs=2, space="PSUM") as xps,
                tc.tile_pool(name="acps", bufs=1, space="PSUM") as aps,
                tc.tile_pool(name="rowps", bufs=2, space="PSUM") as rps,
            ):
                acp = aps.tile([P, 2 * NT], dt.float32, tag="acp", name="acp")
                xgs = []
                for g in range(NT // XG):
                    xxg = xl.tile([P, XG * D], dt.float16, tag=f"xx{g}",
                                  name="xxg")
                    nc.sync.dma_start(
                        xxg[:].rearrange("p (q d) -> p q d", q=XG),
                        x_d.rearrange("(gq p) d -> p gq d",
                                      p=P)[:, g * XG:(g + 1) * XG, :])
                    xgs.append(xxg)
                adj_load(0)
                adj_load(1)
                for it in range(NT):
                    g, gi = divmod(it, XG)
                    xx = xgs[g][:, gi * D:(gi + 1) * D]
                    tp = xps.tile([P, D], dt.float16, tag="tp")
                    for c in range(DC):
                        nc.tensor.transpose(tp[:, c * P:(c + 1) * P],
                                            xx[:, c * P:(c + 1) * P],
                                            ident16[:])
                    xt_dst = xT[:].rearrange("p (c n) -> p c n",
                                             c=DC)[:, :, it * P:(it + 1) * P]
                    xt_src = tp[:].rearrange("p (c q) -> p c q", c=DC)
                    if it % 3 == 1:
                        nc.scalar.copy(xt_dst, xt_src)
                    else:
                        eng = (nc.any, None, nc.gpsimd)[it % 3]
                        eng.tensor_copy(xt_dst, xt_src)
                    for c in range(DC):
                        nc.tensor.matmul(acp[:, it * 2:(it + 1) * 2],
                                         xT[:, c * N + it * P:
                                            c * N + (it + 1) * P],
                                         a_f[:, c * 2:(c + 1) * 2],
                                         start=(c == 0), stop=(c == DC - 1))
                nc.any.tensor_copy(ac_sb[:], acp[:])
                nc.scalar.activation(v1c[:], ac_sb[:, 0:2 * NT:2], AF.Exp,
                                     bias=nln8[:])
                nc.scalar.activation(r_all[:], ac_sb[:, 1:2 * NT:2], AF.Exp,
                                     scale=-0.8)
                # a1 row form + w row (two psum-half phases)
                for hb in range(2):
                    rowp = rps.tile([1, NH], dt.float32, tag="rowp",
                                    name="rowp")
                    for blk in range(2):
                        for c in range(DC):
                            nc.tensor.matmul(
                                rowp[:, blk * 512:(blk + 1) * 512],
                                a_f[:, c * 2:c * 2 + 1],
                                xT[:, c * N + hb * NH + blk * 512:
                                   c * N + hb * NH + (blk + 1) * 512],
                                start=(c == 0), stop=(c == DC - 1))
                    nc.scalar.activation(wrow[:, hb * NH:(hb + 1) * NH],
                                         rowp[:], AF.Exp, scale=-0.8)

            # ---- stage 2: wbc broadcast, v1 diags, h = x @ W, first t
            with (
                tc.tile_pool(name="wbps", bufs=1, space="PSUM") as wps,
                tc.tile_pool(name="hps", bufs=2, space="PSUM") as hps,
            ):
                wbp = wps.tile([P, N], dt.float32, tag="wbp", name="wbp")
                for blk in range(4):
                    nc.tensor.matmul(wbp[:, blk * 512:(blk + 1) * 512],
                                     ones_bf[:],
                                     wrow[:, blk * 512:(blk + 1) * 512],
                                     start=True, stop=True)
                for blk in range(4):
                    nc.scalar.activation(wbc[:, blk * 512:(blk + 1) * 512],
                                         wbp[:, blk * 512:(blk + 1) * 512],
                                         AF.Copy)
                for it in range(NT):
                    eng = nc.vector if it % 2 == 0 else nc.gpsimd
                    eng.tensor_scalar(v1d[it][:], ident16[:],
                                      v1c[:, it:it + 1], None, OP.mult)
                for jp in range(NT // 2):
                    h_ps = hps.tile([P, 2 * D], dt.float32, tag="h")
                    for k in range(2):
                        jt = 2 * jp + k
                        for c in range(DC):
                            nc.tensor.matmul(
                                h_ps[:, k * D:(k + 1) * D],
                                xT[:, c * N + jt * P:c * N + (jt + 1) * P],
                                w_bf[:, c * D:(c + 1) * D],
                                start=(c == 0), stop=(c == DC - 1))
                    nc.scalar.copy(h_all[2 * jp][:], h_ps[:, 0:D])
                    nc.scalar.copy(h_all[2 * jp + 1][:], h_ps[:, D:2 * D])
                # t for jt=0 (bf16, 4x DVE)
                nc.vector.tensor_scalar(t_all[0][:], wbc[:],
                                        r_all[:, 0:1], 1.0, OP.mult, OP.max)

            # ---- stage 3: attention loop; c0/c1 for i-half-0 fused
            with tc.tile_pool(name="hT0", bufs=1, space="PSUM") as h0p:
                hT00 = h0p.tile([P, NH], dt.float32, tag="hT00", name="hT00")
                hT10 = h0p.tile([P, NH], dt.float32, tag="hT10", name="hT10")

                def cmm_ih0(jt):
                    for c, dst in ((0, hT00), (1, hT10)):
                        for nb in range(NH // 512):
                            nc.tensor.matmul(
                                dst[:, nb * 512:(nb + 1) * 512],
                                hs_all[jt][:, c * P:(c + 1) * P],
                                pt_all[jt][:, nb * 512:(nb + 1) * 512],
                                start=(jt == 0), stop=(jt == NT - 1))

                def sm_chain(jt):
                    # rowsum-reduce (Act) -> reciprocal per pair (DVE)
                    nc.scalar.activation(rs_s[jt][:], rsh[jt][:], AF.Identity,
                                         accum_out=rs_p[jt // 2][:,
                                                    jt % 2:jt % 2 + 1])
                    if jt % 2 == 1:
                        nc.vector.reciprocal(rc_p[jt // 2][:],
                                             rs_p[jt // 2][:])
                        for j2 in (jt - 1, jt):
                            nc.scalar.activation(
                                hs_all[j2][:], h_all[j2][:], AF.Copy,
                                scale=rc_p[jt // 2][:, j2 % 2:j2 % 2 + 1])

                with tc.tile_pool(name="qps", bufs=3, space="PSUM") as qp_:
                    for pr in range(NT // 2):
                        if pr >= 1 and pr + 1 < NT // 2:
                            adj_load(pr + 1)
                        adjs = adj_sb[pr % 2]
                        for k in range(2):
                            jt = 2 * pr + k
                            if jt + 1 < NT:
                                nc.vector.tensor_scalar(
                                    t_all[(jt + 1) % 2][:], wbc[:],
                                    r_all[:, jt + 1:jt + 2], 1.0,
                                    OP.mult, OP.max)
                            for q in range(4):
                                qT = qp_.tile([P, NQ], dt.float32, tag="q",
                                              name="qT")
                                for ii in range(4):
                                    it = q * 4 + ii
                                    nc.tensor.matmul(
                                        qT[:, ii * P:(ii + 1) * P],
                                        adjs[:, it * 2 * P + k * P:
                                             it * 2 * P + (k + 1) * P],
                                        v1d[it][:],
                                        start=True, stop=True)
                                qsl = slice(q * NQ, (q + 1) * NQ)
                                eng = (nc.vector, nc.gpsimd)[(q + jt) % 2]
                                eng.scalar_tensor_tensor(
                                    pt_all[jt][:, qsl],
                                    t_all[jt % 2][:, qsl], 1.0,
                                    qT[:], OP.mult, OP.mult,
                                    accum_out=rsh[jt][:, q:q + 1])
                            if jt >= 1:
                                sm_chain(jt - 1)
                            if jt >= 3:
                                cmm_ih0(jt - 3)
                    sm_chain(NT - 1)
                    for jt in range(NT - 3, NT):
                        cmm_ih0(jt)

                # ---- tail
                with tc.tile_pool(name="elu", bufs=4) as ep_, \
                     tc.tile_pool(name="osb", bufs=3) as ob_:

                    def elu_part(src, c, off, width, dve=True):
                        mnneg = ep_.tile([P, width], dt.float16, tag="mn",
                                         name="mnneg")
                        nc.scalar.activation(mnneg[:], src, AF.Relu,
                                             scale=-1.0)
                        em = ep_.tile([P, width], dt.float16, tag="em",
                                      name="em")
                        nc.scalar.activation(em[:], mnneg[:], AF.Exp,
                                             scale=-1.0)
                        rl = ep_.tile([P, width], dt.float16, tag="rl",
                                      name="rl")
                        eng = nc.vector if dve else nc.gpsimd
                        eng.tensor_scalar(rl[:], src, 0.0, None, OP.max)
                        nc.vector.tensor_tensor(A_bf[c][:, off:off + width],
                                                em[:], rl[:], OP.add)

                    with tc.tile_pool(name="hT1a", bufs=1,
                                      space="PSUM") as h1a:
                        hT01 = h1a.tile([P, NH], dt.float32, tag="hT01",
                                        name="hT01")
                        for jt in range(NT):
                            for nb in range(NH // 512):
                                nc.tensor.matmul(
                                    hT01[:, nb * 512:(nb + 1) * 512],
                                    hs_all[jt][:, 0:P],
                                    pt_all[jt][:, NH + nb * 512:
                                           NH + (nb + 1) * 512],
                                    start=(jt == 0), stop=(jt == NT - 1))
                            if jt == 1:
                                elu_part(hT00[:], 0, 0, NH, dve=True)
                            if jt == 5:
                                elu_part(hT10[:], 1, 0, NH, dve=False)
                        elu_part(hT01[:], 0, NH, NH, dve=True)

                    with tc.tile_pool(name="hT1b", bufs=1,
                                      space="PSUM") as h1b, \
                         tc.tile_pool(name="ops", bufs=2,
                                      space="PSUM") as op_:

                        def out_tile(it0):
                            o_ps = op_.tile([P, 2 * D], dt.float32, tag="o",
                                            name="o_ps")
                            for kk in range(2):
                                it = it0 + kk
                                osl = slice(kk * D, (kk + 1) * D)
                                for c in range(DC):
                                    nc.tensor.matmul(
                                        o_ps[:, osl],
                                        A_bf[c][:, it * P:(it + 1) * P],
                                        han_bf[:, c * D:(c + 1) * D],
                                        start=(c == 0), stop=False)
                                nc.tensor.matmul(o_ps[:, osl], ones_f16[:],
                                                 beff_sb[:], start=False,
                                                 stop=True)
                            o_sb = ob_.tile([P, 2 * D], dt.float32, tag="o",
                                            name="o_sb")
                            nc.any.tensor_copy(o_sb[:], o_ps[:])
                            nc.sync.dma_start(
                                out_d.rearrange("(qq p) d -> p qq d",
                                                p=P)[:, it0:it0 + 2, :],
                                o_sb[:].rearrange("p (qq d) -> p qq d", qq=2))

                        hT11 = h1b.tile([P, NH], dt.float32, tag="hT11",
                                        name="hT11")
                        for jt in range(NT):
                            for nb in range(NH // 512):
                                nc.tensor.matmul(
                                    hT11[:, nb * 512:(nb + 1) * 512],
                                    hs_all[jt][:, P:2 * P],
                                    pt_all[jt][:, NH + nb * 512:
                                           NH + (nb + 1) * 512],
                                    start=(jt == 0), stop=(jt == NT - 1))
                            if jt % 2 == 1 and (jt // 2) * 2 < 8:
                                out_tile((jt // 2) * 2)
                        elu_part(hT11[:], 1, NH, NH, dve=False)
                        for it in range(8, NT, 2):
                            out_tile(it)

    nc.compile()
    return nc


def _get_nc():
    global _CACHED_NC
    if _CACHED_NC is None:
        _CACHED_NC = build_nc()
    return _CACHED_NC


def run(inputs, trace=False):
    x = np.asarray(inputs["x"], dtype=np.float32)
    adj = np.asarray(inputs["adj"], dtype=np.int32)
    W = np.asarray(inputs["W"], dtype=np.float32)
    a = np.asarray(inputs["a"], dtype=np.float32)
    han_w = np.asarray(inputs["han_w"], dtype=np.float32)
    han_b = np.asarray(inputs["han_b"], dtype=np.float32)

    B = x.shape[0]
    f16 = np.float16
    a_rs = np.ascontiguousarray(a.reshape(2, D).T).astype(f16)   # [D, 2]
    b_eff = (han_b - han_w.sum(axis=0)).reshape(1, D).astype(f16)
    W16 = W.astype(f16)
    han16 = han_w.astype(f16)

    nc = _get_nc()
    in_maps = [
        {
            "x16": np.ascontiguousarray(x[b]).astype(f16),
            "adj16": adj[b].astype(f16),
            "W16": W16,
            "a_rs": a_rs,
            "han16": han16,
            "b_eff": b_eff,
        }
        for b in range(B)
    ]
    last_err = None
    for attempt in range(3):
        try:
            res = run_bass_kernel_spmd(nc, in_maps, core_ids=list(range(B)),
                                       trace=trace)
            out = np.stack([np.asarray(r["out"]) for r in res.results], axis=0)
            return out, res
        except Exception as e:  # transient NRT/axon execute failures
            last_err = e
            import time as _time
            _time.sleep(3.0 + 5.0 * attempt)
    raise last_err


def kernel(**inputs) -> np.ndarray:
    out, _ = run(inputs, trace=False)
    return out


# revision 13
# speedup vs baseline: 1.0431x; 1.0431x over previous
"""GAT layer kernel for Trainium2, 8 NeuronCores, data-parallel over batch.

Per batch b (one core each):
    h   = x @ W;  a1 = x @ a[:D];  a2 = x @ a[D:]
    e   = leaky_relu(a1[i] + a2[j], 0.2)
    att = softmax over i of where(adj>0, e, -inf)
    out = elu(att-weighted h) @ han_w + han_b

Key reformulation (exact): with z = a1[i] + a2[j],
    exp(leakyrelu(z)) = exp(a2) * v1[i] * max(1, r[j] * w[i])
where v1 = exp(a1), w = exp(-0.8 a1), r = exp(-0.8 a2); the exp(a2)
factor cancels in the softmax column normalization.  Per j-row we need
pt[j,i] = gate[j,i] * v1[i]/8 * max(1, r[j]w[i]) and its row sum.
gate*v1/8 comes out of one PE matmul of the natural-layout adj tile
against diag(v1/8) (transpose + scale + gate in one pass, fp32 PSUM
quarters, ring of 3 banks).  max(1, r[j]*w[i]) is one 4x-mode
tensor_scalar on a bf16 broadcast w tile.  pt and the softmax row sum
come from one scalar_tensor_tensor with accum_out per quarter
(alternating DVE/Pool).  The row-sum division folds into hs = h/rowsum
(computed on Act via per-partition activation scale); elu's "-1" folds
into b_eff which rides the PSUM->SBUF eviction add.  The i-half-0 part
of h'^T = hs^T @ pt accumulates inside the attention loop; i-half-1
sweeps run in the tail, PSUM banks reused via tile tags, interleaved
with elu + out-projection.
"""

import math

import numpy as np

import concourse.bacc as bacc
import concourse.mybir as mybir
from concourse import masks
from concourse.tile import TileContext
from concourse.bass_utils import run_bass_kernel_spmd

P = 128
N = 2048
D = 256
NT = N // P          # 16 node tiles
DC = D // P          # 2 d chunks
NH = N // 2          # i-half size
NQ = N // 4          # i-quarter size
LN8 = math.log(8.0)

dt = mybir.dt
AF = mybir.ActivationFunctionType
OP = mybir.AluOpType

_CACHED_NC = None


def build_nc():
    nc = bacc.Bacc("TRN2", target_bir_lowering=False, debug=False)

    x_d = nc.dram_tensor("x16", [N, D], dt.float16, kind="ExternalInput")
    adj_d = nc.dram_tensor("adj16", [N, N], dt.float16, kind="ExternalInput")
    w_d = nc.dram_tensor("W16", [D, D], dt.float16, kind="ExternalInput")
    a_d = nc.dram_tensor("a_rs", [D, 2], dt.float16, kind="ExternalInput")
    han_d = nc.dram_tensor("han16", [D, D], dt.float16, kind="ExternalInput")
    beff_d = nc.dram_tensor("b_eff", [1, D], dt.float16, kind="ExternalInput")
    out_d = nc.dram_tensor("out", [N, D], dt.float32, kind="ExternalOutput")

    adj_r = adj_d.rearrange("(it p) j -> p it j", p=P)

    with TileContext(nc) as tc:
        with (
            tc.tile_pool(name="const", bufs=1) as cp,
            tc.tile_pool(name="big", bufs=1) as bp,
        ):
            ident16 = cp.tile([P, P], dt.float16)
            masks.make_identity(nc, ident16[:])
            ones_bf = cp.tile([1, P], dt.bfloat16)
            nc.vector.memset(ones_bf[:], 1.0)
            nln8 = cp.tile([P, 1], dt.float32)
            nc.vector.memset(nln8[:], -LN8)

            w_bf = cp.tile([P, DC * D], dt.float16)
            nc.gpsimd.dma_start(w_bf[:].rearrange("p (c d) -> p c d", c=DC),
                                w_d.rearrange("(c p) d -> p c d", p=P))
            han_bf = cp.tile([P, DC * D], dt.float16)
            nc.gpsimd.dma_start(han_bf[:].rearrange("p (c d) -> p c d", c=DC),
                                han_d.rearrange("(c p) d -> p c d", p=P))
            a_f = cp.tile([P, DC * 2], dt.float16)
            nc.gpsimd.dma_start(a_f[:].rearrange("p (c t) -> p c t", c=DC),
                                a_d.rearrange("(c p) t -> p c t", p=P))
            beff_row = cp.tile([1, D], dt.bfloat16)
            nc.gpsimd.dma_start(beff_row[:], beff_d[:])

            # persistent tensors
            xT = bp.tile([P, DC * N], dt.float16, tag="xT", name="xT")
            h_all = [bp.tile([P, D], dt.float16, tag=f"h{i}", name=f"h{i}")
                     for i in range(NT)]
            hs_all = [bp.tile([P, D], dt.float16, tag=f"hs{i}", name=f"hs{i}")
                      for i in range(NT)]
            pt_all = [bp.tile([P, N], dt.float16, tag=f"pt{i}", name=f"pt{i}")
                      for i in range(NT)]
            v1d = [bp.tile([P, P], dt.float16, tag=f"v1d{i}", name=f"v1d{i}")
                   for i in range(NT)]
            t_all = [bp.tile([P, N], dt.bfloat16, tag=f"t{i}", name=f"t{i}")
                     for i in range(2)]
            adj_sb = [bp.tile([P, NT * 2 * P], dt.float16, tag=f"adj{i}",
                              name=f"adj{i}") for i in range(3)]
            wbc = bp.tile([P, N], dt.bfloat16, tag="wbc", name="wbc")
            wrow = bp.tile([1, N], dt.bfloat16, tag="wrow", name="wrow")
            ac_sb = bp.tile([P, 2 * NT], dt.float32, tag="ac", name="ac")
            v1c = bp.tile([P, NT], dt.float32, tag="v1c", name="v1c")
            r_all = bp.tile([P, NT], dt.float32, tag="r", name="r")
            rsh = [bp.tile([P, 4], dt.float32, tag=f"rsh{i}", name=f"rsh{i}")
                   for i in range(NT)]
            rs_s = [bp.tile([P, 4], dt.float32, tag=f"rss{i}", name=f"rss{i}")
                    for i in range(NT)]
            rs_p = [bp.tile([P, 2], dt.float32, tag=f"rsp{i}", name=f"rsp{i}")
                    for i in range(NT // 2)]
            rc_p = [bp.tile([P, 2], dt.float32, tag=f"rcp{i}", name=f"rcp{i}")
                    for i in range(NT // 2)]
            beffbc = bp.tile([P, 2 * D], dt.float32, tag="bbc", name="bbc")
            A_bf = [bp.tile([P, N], dt.float16, tag=f"A{c}", name=f"A{c}")
                    for c in range(DC)]

            def adj_load(pr):
                nc.sync.dma_start(
                    adj_sb[pr % 3][:].rearrange("p (it j) -> p it j", j=2 * P),
                    adj_r[:, :, pr * 2 * P:(pr + 1) * 2 * P])

            # ---- stage 1: x load/transpose; a-matmuls; exps; wbc; h
            XG = 4
            with (
                tc.tile_pool(name="xload", bufs=1) as xl,
                tc.tile_pool(name="xps", bufs=2, space="PSUM") as xps,
                tc.tile_pool(name="acps", bufs=1, space="PSUM") as aps,
                tc.tile_pool(name="rowps", bufs=2, space="PSUM") as rps,
            ):
                acp = aps.tile([P, 2 * NT], dt.float32, tag="acp", name="acp")
                xgs = []
                for g in range(NT // XG):
                    xxg = xl.tile([P, XG * D], dt.float16, tag=f"xx{g}",
                                  name="xxg")
                    nc.sync.dma_start(
                        xxg[:].rearrange("p (q d) -> p q d", q=XG),
                        x_d.rearrange("(gq p) d -> p gq d",
                                      p=P)[:, g * XG:(g + 1) * XG, :])
                    xgs.append(xxg)
                adj_load(0)
                adj_load(1)
                adj_load(2)
                for it in range(NT):
                    g, gi = divmod(it, XG)
                    xx = xgs[g][:, gi * D:(gi + 1) * D]
                    tp = xps.tile([P, D], dt.float16, tag="tp")
                    for c in range(DC):
                        nc.tensor.transpose(tp[:, c * P:(c + 1) * P],
                                            xx[:, c * P:(c + 1) * P],
                                            ident16[:])
                    xt_dst = xT[:].rearrange("p (c n) -> p c n",
                                             c=DC)[:, :, it * P:(it + 1) * P]
                    xt_src = tp[:].rearrange("p (c q) -> p c q", c=DC)
                    if it % 3 == 1:
                        nc.scalar.copy(xt_dst, xt_src)
                    else:
                        eng = (nc.any, None, nc.gpsimd)[it % 3]
                        eng.tensor_copy(xt_dst, xt_src)
                # a columns (after all copies; xT is one tile anyway)
                for it in range(NT):
                    for c in range(DC):
                        nc.tensor.matmul(acp[:, it * 2:(it + 1) * 2],
                                         xT[:, c * N + it * P:
                                            c * N + (it + 1) * P],
                                         a_f[:, c * 2:(c + 1) * 2],
                                         start=(c == 0), stop=(c == DC - 1))
                nc.any.tensor_copy(ac_sb[:], acp[:])
                nc.scalar.activation(v1c[:], ac_sb[:, 0:2 * NT:2], AF.Exp,
                                     bias=nln8[:])
                nc.scalar.activation(r_all[:], ac_sb[:, 1:2 * NT:2], AF.Exp,
                                     scale=-0.8)
                for it in range(NT):
                    eng = nc.vector if it % 2 == 0 else nc.gpsimd
                    eng.tensor_scalar(v1d[it][:], ident16[:],
                                      v1c[:, it:it + 1], None, OP.mult)
                # a1 row form + w row (two psum-half phases)
                for hb in range(2):
                    rowp = rps.tile([1, NH], dt.float32, tag="rowp",
                                    name="rowp")
                    for blk in range(2):
                        for c in range(DC):
                            nc.tensor.matmul(
                                rowp[:, blk * 512:(blk + 1) * 512],
                                a_f[:, c * 2:c * 2 + 1],
                                xT[:, c * N + hb * NH + blk * 512:
                                   c * N + hb * NH + (blk + 1) * 512],
                                start=(c == 0), stop=(c == DC - 1))
                    nc.scalar.activation(wrow[:, hb * NH:(hb + 1) * NH],
                                         rowp[:], AF.Exp, scale=-0.8)

            with (
                tc.tile_pool(name="wbps", bufs=1, space="PSUM") as wps,
                tc.tile_pool(name="hps", bufs=2, space="PSUM") as hps,
            ):
                wbp = wps.tile([P, N], dt.float32, tag="wbp", name="wbp")
                for blk in range(4):
                    nc.tensor.matmul(wbp[:, blk * 512:(blk + 1) * 512],
                                     ones_bf[:],
                                     wrow[:, blk * 512:(blk + 1) * 512],
                                     start=True, stop=True)
                for blk in range(4):
                    nc.scalar.activation(wbc[:, blk * 512:(blk + 1) * 512],
                                         wbp[:, blk * 512:(blk + 1) * 512],
                                         AF.Copy)
                # t for jt=0 as soon as wbc exists
                nc.vector.tensor_scalar(t_all[0][:], wbc[:],
                                        r_all[:, 0:1], 1.0, OP.mult, OP.max)
                # beff broadcast [P, 2D] via PE outer product
                bps = wps.tile([P, 2 * D], dt.float32, tag="bps", name="bps")
                for kk in range(2):
                    nc.tensor.matmul(bps[:, kk * D:(kk + 1) * D], ones_bf[:],
                                     beff_row[:], start=True, stop=True)
                nc.gpsimd.tensor_copy(beffbc[:], bps[:])
                # h = x @ W
                for jp in range(NT // 2):
                    h_ps = hps.tile([P, 2 * D], dt.float32, tag="h")
                    for k in range(2):
                        jt = 2 * jp + k
                        for c in range(DC):
                            nc.tensor.matmul(
                                h_ps[:, k * D:(k + 1) * D],
                                xT[:, c * N + jt * P:c * N + (jt + 1) * P],
                                w_bf[:, c * D:(c + 1) * D],
                                start=(c == 0), stop=(c == DC - 1))
                    for k in range(2):
                        jt = 2 * jp + k
                        src = h_ps[:, k * D:(k + 1) * D]
                        m = (jp * 2 + k) % 3
                        if m == 1:
                            nc.scalar.copy(h_all[jt][:], src)
                        else:
                            eng = (nc.any, None, nc.gpsimd)[m]
                            eng.tensor_copy(h_all[jt][:], src)

            # ---- stage 3: attention loop; c0/c1 for i-half-0 fused
            with tc.tile_pool(name="hT", bufs=1, space="PSUM") as h0p:
                hTa = h0p.tile([P, NH], dt.float32, tag="hTa", name="hTa")
                hTb = h0p.tile([P, NH], dt.float32, tag="hTb", name="hTb")

                def cmm(jt, dsts, ioff):
                    for c, dst in ((0, dsts[0]), (1, dsts[1])):
                        for nb in range(NH // 512):
                            nc.tensor.matmul(
                                dst[:, nb * 512:(nb + 1) * 512],
                                hs_all[jt][:, c * P:(c + 1) * P],
                                pt_all[jt][:, ioff + nb * 512:
                                       ioff + (nb + 1) * 512],
                                start=(jt == 0), stop=(jt == NT - 1))

                def sm_chain(jt):
                    nc.scalar.activation(rs_s[jt][:], rsh[jt][:], AF.Identity,
                                         accum_out=rs_p[jt // 2][:,
                                                    jt % 2:jt % 2 + 1])
                    if jt % 2 == 1:
                        nc.vector.reciprocal(rc_p[jt // 2][:],
                                             rs_p[jt // 2][:])
                        for j2 in (jt - 1, jt):
                            nc.scalar.activation(
                                hs_all[j2][:], h_all[j2][:], AF.Copy,
                                scale=rc_p[jt // 2][:, j2 % 2:j2 % 2 + 1])

                with tc.tile_pool(name="qps", bufs=3, space="PSUM") as qp_:
                    for pr in range(NT // 2):
                        if 0 <= pr < NT // 2 - 3:
                            adj_load(pr + 3)
                        adjs = adj_sb[pr % 3]
                        for k in range(2):
                            jt = 2 * pr + k
                            if jt + 1 < NT:
                                nc.vector.tensor_scalar(
                                    t_all[(jt + 1) % 2][:], wbc[:],
                                    r_all[:, jt + 1:jt + 2], 1.0,
                                    OP.mult, OP.max)
                            for q in range(4):
                                qT = qp_.tile([P, NQ], dt.float32, tag="q",
                                              name="qT")
                                for ii in range(4):
                                    it = q * 4 + ii
                                    nc.tensor.matmul(
                                        qT[:, ii * P:(ii + 1) * P],
                                        adjs[:, it * 2 * P + k * P:
                                             it * 2 * P + (k + 1) * P],
                                        v1d[it][:],
                                        start=True, stop=True)
                                qsl = slice(q * NQ, (q + 1) * NQ)
                                eng = (nc.vector, nc.gpsimd)[(q + jt) % 2]
                                eng.scalar_tensor_tensor(
                                    pt_all[jt][:, qsl],
                                    t_all[jt % 2][:, qsl], 1.0,
                                    qT[:], OP.mult, OP.mult,
                                    accum_out=rsh[jt][:, q:q + 1])
                            if jt >= 1:
                                sm_chain(jt - 1)
                            if jt >= 3:
                                cmm(jt - 3, (hTa, hTb), 0)
                    sm_chain(NT - 1)
                    for jt in range(NT - 3, NT):
                        cmm(jt, (hTa, hTb), 0)

                # ---- tail: i-half-1 sweeps with psum tag reuse
                with (
                    tc.tile_pool(name="elu", bufs=4) as ep_,
                    tc.tile_pool(name="osb", bufs=3) as ob_,
                    tc.tile_pool(name="ops", bufs=2, space="PSUM") as op_,
                ):
                    def elu_part(src, c, off, width, dve=True):
                        mnneg = ep_.tile([P, width], dt.float16, tag="mn",
                                         name="mnneg")
                        nc.scalar.activation(mnneg[:], src, AF.Relu,
                                             scale=-1.0)
                        em = ep_.tile([P, width], dt.float16, tag="em",
                                      name="em")
                        nc.scalar.activation(em[:], mnneg[:], AF.Exp,
                                             scale=-1.0)
                        rl = ep_.tile([P, width], dt.float16, tag="rl",
                                      name="rl")
                        eng = nc.vector if dve else nc.gpsimd
                        eng.tensor_scalar(rl[:], src, 0.0, None, OP.max)
                        nc.vector.tensor_tensor(A_bf[c][:, off:off + width],
                                                em[:], rl[:], OP.add)

                    def out_tile(it0):
                        o_ps = op_.tile([P, 2 * D], dt.float32, tag="o",
                                        name="o_ps")
                        for kk in range(2):
                            it = it0 + kk
                            osl = slice(kk * D, (kk + 1) * D)
                            for c in range(DC):
                                nc.tensor.matmul(
                                    o_ps[:, osl],
                                    A_bf[c][:, it * P:(it + 1) * P],
                                    han_bf[:, c * D:(c + 1) * D],
                                    start=(c == 0), stop=(c == DC - 1))
                        o_sb = ob_.tile([P, 2 * D], dt.float32, tag="o",
                                        name="o_sb")
                        nc.vector.tensor_tensor(o_sb[:], o_ps[:], beffbc[:],
                                                OP.add)
                        nc.sync.dma_start(
                            out_d.rearrange("(qq p) d -> p qq d",
                                            p=P)[:, it0:it0 + 2, :],
                            o_sb[:].rearrange("p (qq d) -> p qq d", qq=2))

                    # elu of i-half-0 (frees hTa/hTb for tag reuse)
                    elu_part(hTa[:], 0, 0, NH, dve=True)
                    elu_part(hTb[:], 1, 0, NH, dve=False)

                    # c0-ih1 sweep into fresh banks (from qps), c1-ih1 into
                    # reused hTa banks; out tiles interleave as A fills in.
                    with tc.tile_pool(name="hT1", bufs=1,
                                      space="PSUM") as h1p:
                        hTc = h1p.tile([P, NH], dt.float32, tag="hTc",
                                       name="hTc")
                        for jt in range(NT):
                            for nb in range(NH // 512):
                                nc.tensor.matmul(
                                    hTc[:, nb * 512:(nb + 1) * 512],
                                    hs_all[jt][:, 0:P],
                                    pt_all[jt][:, NH + nb * 512:
                                           NH + (nb + 1) * 512],
                                    start=(jt == 0), stop=(jt == NT - 1))
                            if jt == 7:
                                out_tile(0)
                            if jt == 11:
                                out_tile(2)
                        elu_part(hTc[:], 0, NH, NH, dve=True)
                        hTd = h0p.tile([P, NH], dt.float32, tag="hTa",
                                       name="hTd")
                        for jt in range(NT):
                            for nb in range(NH // 512):
                                nc.tensor.matmul(
                                    hTd[:, nb * 512:(nb + 1) * 512],
                                    hs_all[jt][:, P:2 * P],
                                    pt_all[jt][:, NH + nb * 512:
                                           NH + (nb + 1) * 512],
                                    start=(jt == 0), stop=(jt == NT - 1))
                            if jt == 7:
                                out_tile(4)
                            if jt == 11:
                                out_tile(6)
                        # A[c1] i-half-1 in two chunks to unblock out tiles
                        elu_part(hTd[:, 0:512], 1, NH, 512, dve=False)
                        out_tile(8)
                        out_tile(10)
                        elu_part(hTd[:, 512:NH], 1, NH + 512, 512, dve=False)
                        out_tile(12)
                        out_tile(14)

    nc.compile()
    return nc


def _get_nc():
    global _CACHED_NC
    if _CACHED_NC is None:
        _CACHED_NC = build_nc()
    return _CACHED_NC


def run(inputs, trace=False):
    x = np.asarray(inputs["x"], dtype=np.float32)
    adj = np.asarray(inputs["adj"], dtype=np.int32)
    W = np.asarray(inputs["W"], dtype=np.float32)
    a = np.asarray(inputs["a"], dtype=np.float32)
    han_w = np.asarray(inputs["han_w"], dtype=np.float32)
    han_b = np.asarray(inputs["han_b"], dtype=np.float32)

    B = x.shape[0]
    f16 = np.float16
    a_rs = np.ascontiguousarray(a.reshape(2, D).T).astype(f16)   # [D, 2]
    b_eff = (han_b - han_w.sum(axis=0)).reshape(1, D).astype(f16)
    W16 = W.astype(f16)
    han16 = han_w.astype(f16)

    nc = _get_nc()
    in_maps = [
        {
            "x16": np.ascontiguousarray(x[b]).astype(f16),
            "adj16": adj[b].astype(f16),
            "W16": W16,
            "a_rs": a_rs,
            "han16": han16,
            "b_eff": b_eff,
        }
        for b in range(B)
    ]
    last_err = None
    for attempt in range(3):
        try:
            res = run_bass_kernel_spmd(nc, in_maps, core_ids=list(range(B)),
                                       trace=trace)
            out = np.stack([np.asarray(r["out"]) for r in res.results], axis=0)
            return out, res
        except Exception as e:  # transient NRT/axon execute failures
            last_err = e
            import time as _time
            _time.sleep(3.0 + 5.0 * attempt)
    raise last_err


def kernel(**inputs) -> np.ndarray:
    out, _ = run(inputs, trace=False)
    return out


# revision 15
# speedup vs baseline: 1.1277x; 1.0810x over previous
"""GAT layer kernel for Trainium2, 8 NeuronCores, data-parallel over batch.

Per batch b (one core each):
    h   = x @ W;  a1 = x @ a[:D];  a2 = x @ a[D:]
    e   = leaky_relu(a1[i] + a2[j], 0.2)
    att = softmax over i of where(adj>0, e, -inf)
    out = elu(att-weighted h) @ han_w + han_b

Key reformulation (exact): with z = a1[i] + a2[j],
    exp(leakyrelu(z)) = exp(a2) * v1[i] * max(1, r[j] * w[i])
where v1 = exp(a1), w = exp(-0.8 a1), r = exp(-0.8 a2); the exp(a2)
factor cancels in the softmax column normalization.  Per j-row we need
pt[j,i] = gate[j,i] * v1[i]/8 * max(1, r[j]w[i]) and its row sum.
gate*v1/8 comes out of one PE matmul of the natural-layout adj tile
against diag(v1/8) (transpose + scale + gate in one pass, fp32 PSUM
quarters, ring of 3 banks).  max(1, r[j]*w[i]) is one 4x-mode
tensor_scalar on a bf16 broadcast w tile.  pt and the softmax row sum
come from one scalar_tensor_tensor with accum_out per quarter
(alternating DVE/Pool).  The row-sum division folds into hs = h/rowsum
(computed on Act via per-partition activation scale); elu's "-1" folds
into b_eff which rides the PSUM->SBUF eviction add.  The i-half-0 part
of h'^T = hs^T @ pt accumulates inside the attention loop; i-half-1
sweeps run in the tail, PSUM banks reused via tile tags, interleaved
with elu + out-projection.
"""

import math

import numpy as np

import concourse.bacc as bacc
import concourse.mybir as mybir
from concourse import masks
from concourse.tile import TileContext
from concourse.bass_utils import run_bass_kernel_spmd

P = 128
N = 2048
D = 256
NT = N // P          # 16 node tiles
DC = D // P          # 2 d chunks
NH = N // 2          # i-half size
NQ = N // 4          # i-quarter size
LN8 = math.log(8.0)

dt = mybir.dt
AF = mybir.ActivationFunctionType
OP = mybir.AluOpType

_CACHED_NC = None


def build_nc():
    nc = bacc.Bacc("TRN2", target_bir_lowering=False, debug=False)

    x_d = nc.dram_tensor("x16", [N, D], dt.float16, kind="ExternalInput")
    adj_d = nc.dram_tensor("adj16", [N, N], dt.float16, kind="ExternalInput")
    w_d = nc.dram_tensor("W16", [D, D], dt.float16, kind="ExternalInput")
    a_d = nc.dram_tensor("a_rs", [D, 2], dt.float16, kind="ExternalInput")
    han_d = nc.dram_tensor("han16", [D, D], dt.float16, kind="ExternalInput")
    beff_d = nc.dram_tensor("b_eff", [1, D], dt.bfloat16, kind="ExternalInput")
    out_d = nc.dram_tensor("out", [N, D], dt.float32, kind="ExternalOutput")

    adj_r = adj_d.rearrange("(it p) j -> p it j", p=P)

    with TileContext(nc) as tc:
        with (
            tc.tile_pool(name="const", bufs=1) as cp,
            tc.tile_pool(name="big", bufs=1) as bp,
        ):
            ident16 = cp.tile([P, P], dt.float16)
            masks.make_identity(nc, ident16[:])
            ones_bf = cp.tile([1, P], dt.bfloat16)
            nc.vector.memset(ones_bf[:], 1.0)
            nln8 = cp.tile([P, 1], dt.float32)
            nc.vector.memset(nln8[:], -LN8)

            w_bf = cp.tile([P, DC * D], dt.float16)
            han_bf = cp.tile([P, DC * D], dt.float16)
            a_f = cp.tile([P, DC * 2], dt.float16)
            beff_row = cp.tile([1, D], dt.bfloat16)

            def param_loads():
                nc.sync.dma_start(
                    a_f[:].rearrange("p (c t) -> p c t", c=DC),
                    a_d.rearrange("(c p) t -> p c t", p=P))
                nc.sync.dma_start(
                    w_bf[:].rearrange("p (c d) -> p c d", c=DC),
                    w_d.rearrange("(c p) d -> p c d", p=P))
                nc.sync.dma_start(
                    han_bf[:].rearrange("p (c d) -> p c d", c=DC),
                    han_d.rearrange("(c p) d -> p c d", p=P))
                nc.sync.dma_start(beff_row[:], beff_d[:])

            # persistent tensors
            xT = bp.tile([P, DC * N], dt.float16, tag="xT", name="xT")
            h_all = [bp.tile([P, D], dt.float16, tag=f"h{i}", name=f"h{i}")
                     for i in range(NT)]
            hs_all = [bp.tile([P, D], dt.float16, tag=f"hs{i}", name=f"hs{i}")
                      for i in range(NT)]
            pt_all = [[bp.tile([P, NH], dt.float16, tag=f"pt{i}_{hh}",
                                name=f"pt{i}_{hh}") for hh in range(2)]
                      for i in range(NT)]
            v1d = [bp.tile([P, P], dt.float16, tag=f"v1d{i}", name=f"v1d{i}")
                   for i in range(NT)]
            t_all = [bp.tile([P, N], dt.bfloat16, tag=f"t{i}", name=f"t{i}")
                     for i in range(2)]
            adj_sb = [bp.tile([P, NT * 2 * P], dt.float16, tag=f"adj{i}",
                              name=f"adj{i}") for i in range(3)]
            wbc = bp.tile([P, N], dt.bfloat16, tag="wbc", name="wbc")
            wrow = bp.tile([1, N], dt.bfloat16, tag="wrow", name="wrow")
            ac_sb = bp.tile([P, 2 * NT], dt.float32, tag="ac", name="ac")
            v1c = bp.tile([P, NT], dt.float32, tag="v1c", name="v1c")
            r_all = bp.tile([P, NT], dt.float32, tag="r", name="r")
            rsh = [[bp.tile([P, 2], dt.float32, tag=f"rsh{i}_{hh}",
                             name=f"rsh{i}_{hh}") for hh in range(2)]
                   for i in range(NT)]
            rs_s = [bp.tile([P, 2], dt.float32, tag=f"rss{i}", name=f"rss{i}")
                    for i in range(NT)]
            sdp = [bp.tile([P, 2], dt.float32, tag=f"sdp{i}", name=f"sdp{i}")
                   for i in range(NT)]
            rs_p = [bp.tile([P, 2], dt.float32, tag=f"rsp{i}", name=f"rsp{i}")
                    for i in range(NT // 2)]
            rc_p = [bp.tile([P, 2], dt.float32, tag=f"rcp{i}", name=f"rcp{i}")
                    for i in range(NT // 2)]
            beffbc = bp.tile([P, 2 * D], dt.float32, tag="bbc", name="bbc")
            A_bf = [bp.tile([P, N], dt.float16, tag=f"A{c}", name=f"A{c}")
                    for c in range(DC)]

            def adj_load(pr):
                nc.sync.dma_start(
                    adj_sb[pr % 3][:].rearrange("p (it j) -> p it j", j=2 * P),
                    adj_r[:, :, pr * 2 * P:(pr + 1) * 2 * P])

            # ---- stage 1: x load/transpose; a-matmuls; exps; wbc; h
            XG = 4
            with (
                tc.tile_pool(name="xload", bufs=1) as xl,
                tc.tile_pool(name="xps", bufs=2, space="PSUM") as xps,
                tc.tile_pool(name="acps", bufs=1, space="PSUM") as aps,
                tc.tile_pool(name="rowps", bufs=2, space="PSUM") as rps,
            ):
                acp = aps.tile([P, 2 * NT], dt.float32, tag="acp", name="acp")
                xgs = []
                for g in range(NT // XG):
                    xxg = xl.tile([P, XG * D], dt.float16, tag=f"xx{g}",
                                  name="xxg")
                    nc.sync.dma_start(
                        xxg[:].rearrange("p (q d) -> p q d", q=XG),
                        x_d.rearrange("(gq p) d -> p gq d",
                                      p=P)[:, g * XG:(g + 1) * XG, :])
                    xgs.append(xxg)
                param_loads()
                adj_load(0)
                adj_load(1)
                adj_load(2)
                for g in range(NT // XG):
                    tp = xps.tile([P, XG * D], dt.float16, tag="tp")
                    for gi in range(XG):
                        it = g * XG + gi
                        xx = xgs[g][:, gi * D:(gi + 1) * D]
                        for c in range(DC):
                            nc.tensor.transpose(
                                tp[:, gi * D + c * P:gi * D + (c + 1) * P],
                                xx[:, c * P:(c + 1) * P], ident16[:])
                    # one copy per 4-it group: [p][c][it'][128]
                    xt_dst = xT[:].rearrange(
                        "p (c gg n) -> p c gg n", c=DC,
                        gg=NT // XG)[:, :, g, :]
                    xt_src = tp[:].rearrange("p (gi c q) -> p c gi q",
                                             gi=XG, c=DC)
                    nc.vector.tensor_copy(
                        xt_dst.rearrange("p c (gi q) -> p c gi q", gi=XG),
                        xt_src)
                # a columns (after all copies; xT is one tile anyway)
                for it in range(NT):
                    for c in range(DC):
                        nc.tensor.matmul(acp[:, it * 2:(it + 1) * 2],
                                         xT[:, c * N + it * P:
                                            c * N + (it + 1) * P],
                                         a_f[:, c * 2:(c + 1) * 2],
                                         start=(c == 0), stop=(c == DC - 1))
                nc.any.tensor_copy(ac_sb[:], acp[:])
                nc.scalar.activation(v1c[:], ac_sb[:, 0:2 * NT:2], AF.Exp,
                                     bias=nln8[:])
                nc.scalar.activation(r_all[:], ac_sb[:, 1:2 * NT:2], AF.Exp,
                                     scale=-0.8)
                for it in range(NT):
                    eng = nc.vector if it % 2 == 0 else nc.gpsimd
                    eng.tensor_scalar(v1d[it][:], ident16[:],
                                      v1c[:, it:it + 1], None, OP.mult)
                # a1 row form + w row (two psum-half phases)
                for hb in range(2):
                    rowp = rps.tile([1, NH], dt.float32, tag="rowp",
                                    name="rowp")
                    for blk in range(2):
                        for c in range(DC):
                            nc.tensor.matmul(
                                rowp[:, blk * 512:(blk + 1) * 512],
                                a_f[:, c * 2:c * 2 + 1],
                                xT[:, c * N + hb * NH + blk * 512:
                                   c * N + hb * NH + (blk + 1) * 512],
                                start=(c == 0), stop=(c == DC - 1))
                    nc.scalar.activation(wrow[:, hb * NH:(hb + 1) * NH],
                                         rowp[:], AF.Exp, scale=-0.8)

            with (
                tc.tile_pool(name="wbps", bufs=1, space="PSUM") as wps,
                tc.tile_pool(name="hps", bufs=2, space="PSUM") as hps,
            ):
                wbp = wps.tile([P, N], dt.float32, tag="wbp", name="wbp")
                for blk in range(4):
                    nc.tensor.matmul(wbp[:, blk * 512:(blk + 1) * 512],
                                     ones_bf[:],
                                     wrow[:, blk * 512:(blk + 1) * 512],
                                     start=True, stop=True)
                for blk in range(4):
                    nc.scalar.activation(wbc[:, blk * 512:(blk + 1) * 512],
                                         wbp[:, blk * 512:(blk + 1) * 512],
                                         AF.Copy)
                # t for jt=0 as soon as wbc exists
                nc.vector.tensor_scalar(t_all[0][:], wbc[:],
                                        r_all[:, 0:1], 1.0, OP.mult, OP.max)
                # beff broadcast [P, 2D] via PE outer product
                bps = wps.tile([P, 2 * D], dt.float32, tag="bps", name="bps")
                for kk in range(2):
                    nc.tensor.matmul(bps[:, kk * D:(kk + 1) * D], ones_bf[:],
                                     beff_row[:], start=True, stop=True)
                nc.gpsimd.tensor_copy(beffbc[:], bps[:])
                # h = x @ W
                for jp in range(NT // 2):
                    h_ps = hps.tile([P, 2 * D], dt.float32, tag="h")
                    for k in range(2):
                        jt = 2 * jp + k
                        for c in range(DC):
                            nc.tensor.matmul(
                                h_ps[:, k * D:(k + 1) * D],
                                xT[:, c * N + jt * P:c * N + (jt + 1) * P],
                                w_bf[:, c * D:(c + 1) * D],
                                start=(c == 0), stop=(c == DC - 1))
                    for k in range(2):
                        jt = 2 * jp + k
                        src = h_ps[:, k * D:(k + 1) * D]
                        m = (jp * 2 + k) % 3
                        if m == 1:
                            nc.scalar.copy(h_all[jt][:], src)
                        else:
                            eng = (nc.any, None, nc.gpsimd)[m]
                            eng.tensor_copy(h_all[jt][:], src)

            # ---- stage 3: attention loop; c0/c1 for i-half-0 fused
            with tc.tile_pool(name="hT", bufs=1, space="PSUM") as h0p:
                hTa = h0p.tile([P, NH], dt.float32, tag="hTa", name="hTa")
                hTb = h0p.tile([P, NH], dt.float32, tag="hTb", name="hTb")

                def cmm(jt, dsts):
                    for c, dst in ((0, dsts[0]), (1, dsts[1])):
                        for nb in range(NH // 512):
                            nc.tensor.matmul(
                                dst[:, nb * 512:(nb + 1) * 512],
                                hs_all[jt][:, c * P:(c + 1) * P],
                                pt_all[jt][0][:, nb * 512:(nb + 1) * 512],
                                start=(jt == 0), stop=(jt == NT - 1))

                def sm_chain(jt):
                    for hh in range(2):
                        nc.scalar.activation(rs_s[jt][:], rsh[jt][hh][:],
                                             AF.Identity,
                                             accum_out=sdp[jt][:, hh:hh + 1])
                    nc.vector.tensor_scalar(rs_p[jt // 2][:,
                                            jt % 2:jt % 2 + 1],
                                            sdp[jt][:, 0:1],
                                            sdp[jt][:, 1:2], None, OP.add)
                    if jt % 2 == 1:
                        nc.vector.reciprocal(rc_p[jt // 2][:],
                                             rs_p[jt // 2][:])
                        for j2 in (jt - 1, jt):
                            nc.scalar.activation(
                                hs_all[j2][:], h_all[j2][:], AF.Copy,
                                scale=rc_p[jt // 2][:, j2 % 2:j2 % 2 + 1])

                with tc.tile_pool(name="qps", bufs=3, space="PSUM") as qp_:
                    for pr in range(NT // 2):
                        if 0 <= pr < NT // 2 - 3:
                            adj_load(pr + 3)
                        adjs = adj_sb[pr % 3]
                        for k in range(2):
                            jt = 2 * pr + k
                            if jt + 1 < NT:
                                nc.vector.tensor_scalar(
                                    t_all[(jt + 1) % 2][:], wbc[:],
                                    r_all[:, jt + 1:jt + 2], 1.0,
                                    OP.mult, OP.max)
                            for q in range(4):
                                qT = qp_.tile([P, NQ], dt.float32, tag="q",
                                              name="qT")
                                for ii in range(4):
                                    it = q * 4 + ii
                                    nc.tensor.matmul(
                                        qT[:, ii * P:(ii + 1) * P],
                                        adjs[:, it * 2 * P + k * P:
                                             it * 2 * P + (k + 1) * P],
                                        v1d[it][:],
                                        start=True, stop=True)
                                hh, qh = divmod(q, 2)
                                qsl = slice(q * NQ, (q + 1) * NQ)
                                psl = slice(qh * NQ, (qh + 1) * NQ)
                                eng = (nc.vector, nc.gpsimd)[hh]
                                eng.scalar_tensor_tensor(
                                    pt_all[jt][hh][:, psl],
                                    t_all[jt % 2][:, qsl], 1.0,
                                    qT[:], OP.mult, OP.mult,
                                    accum_out=rsh[jt][hh][:, qh:qh + 1])
                            if jt >= 1:
                                sm_chain(jt - 1)
                            if jt >= 3:
                                cmm(jt - 3, (hTa, hTb))
                    sm_chain(NT - 1)
                    for jt in range(NT - 3, NT):
                        cmm(jt, (hTa, hTb))

                # ---- tail: i-half-1 sweeps with psum tag reuse
                with (
                    tc.tile_pool(name="elu", bufs=4) as ep_,
                    tc.tile_pool(name="osb", bufs=3) as ob_,
                    tc.tile_pool(name="ops", bufs=2, space="PSUM") as op_,
                ):
                    def elu_part(src, c, off, width, dve=True):
                        mnneg = ep_.tile([P, width], dt.float16, tag="mn",
                                         name="mnneg")
                        nc.scalar.activation(mnneg[:], src, AF.Relu,
                                             scale=-1.0)
                        em = ep_.tile([P, width], dt.float16, tag="em",
                                      name="em")
                        nc.scalar.activation(em[:], mnneg[:], AF.Exp,
                                             scale=-1.0)
                        rl = ep_.tile([P, width], dt.float16, tag="rl",
                                      name="rl")
                        eng = nc.vector if dve else nc.gpsimd
                        eng.tensor_scalar(rl[:], src, 0.0, None, OP.max)
                        nc.vector.tensor_tensor(A_bf[c][:, off:off + width],
                                                em[:], rl[:], OP.add)

                    def out_tile(it0):
                        o_ps = op_.tile([P, 2 * D], dt.float32, tag="o",
                                        name="o_ps")
                        for kk in range(2):
                            it = it0 + kk
                            osl = slice(kk * D, (kk + 1) * D)
                            for c in range(DC):
                                nc.tensor.matmul(
                                    o_ps[:, osl],
                                    A_bf[c][:, it * P:(it + 1) * P],
                                    han_bf[:, c * D:(c + 1) * D],
                                    start=(c == 0), stop=(c == DC - 1))
                        o_sb = ob_.tile([P, 2 * D], dt.float32, tag="o",
                                        name="o_sb")
                        nc.vector.tensor_tensor(o_sb[:], o_ps[:], beffbc[:],
                                                OP.add)
                        nc.sync.dma_start(
                            out_d.rearrange("(qq p) d -> p qq d",
                                            p=P)[:, it0:it0 + 2, :],
                            o_sb[:].rearrange("p (qq d) -> p qq d", qq=2))

                    # elu of i-half-0 (frees hTa/hTb for tag reuse)
                    elu_part(hTa[:], 0, 0, NH, dve=True)
                    elu_part(hTb[:], 1, 0, NH, dve=False)

                    # c0-ih1 sweep into fresh banks (from qps), c1-ih1 into
                    # reused hTa banks; out tiles interleave as A fills in.
                    with tc.tile_pool(name="hT1", bufs=1,
                                      space="PSUM") as h1p:
                        hTc = h1p.tile([P, NH], dt.float32, tag="hTc",
                                       name="hTc")
                        for jt in range(NT):
                            for nb in range(NH // 512):
                                nc.tensor.matmul(
                                    hTc[:, nb * 512:(nb + 1) * 512],
                                    hs_all[jt][:, 0:P],
                                    pt_all[jt][1][:, nb * 512:
                                           (nb + 1) * 512],
                                    start=(jt == 0), stop=(jt == NT - 1))
                            if jt == 7:
                                out_tile(0)
                            if jt == 11:
                                out_tile(2)
                        elu_part(hTc[:], 0, NH, NH, dve=True)
                        hTd = h0p.tile([P, NH], dt.float32, tag="hTa",
                                       name="hTd")
                        for jt in range(NT):
                            for nb in range(NH // 512):
                                nc.tensor.matmul(
                                    hTd[:, nb * 512:(nb + 1) * 512],
                                    hs_all[jt][:, P:2 * P],
                                    pt_all[jt][1][:, nb * 512:
                                           (nb + 1) * 512],
                                    start=(jt == 0), stop=(jt == NT - 1))
                            if jt == 7:
                                out_tile(4)
                            if jt == 11:
                                out_tile(6)
                        # A[c1] i-half-1 in two chunks to unblock out tiles
                        elu_part(hTd[:, 0:512], 1, NH, 512, dve=False)
                        out_tile(8)
                        out_tile(10)
                        elu_part(hTd[:, 512:NH], 1, NH + 512, 512, dve=False)
                        out_tile(12)
                        out_tile(14)

    nc.compile()
    return nc


def _get_nc():
    global _CACHED_NC
    if _CACHED_NC is None:
        _CACHED_NC = build_nc()
    return _CACHED_NC


def run(inputs, trace=False):
    x = np.asarray(inputs["x"], dtype=np.float32)
    adj = np.asarray(inputs["adj"], dtype=np.int32)
    W = np.asarray(inputs["W"], dtype=np.float32)
    a = np.asarray(inputs["a"], dtype=np.float32)
    han_w = np.asarray(inputs["han_w"], dtype=np.float32)
    han_b = np.asarray(inputs["han_b"], dtype=np.float32)

    B = x.shape[0]
    f16 = np.float16
    a_rs = np.ascontiguousarray(a.reshape(2, D).T).astype(f16)   # [D, 2]
    import ml_dtypes
    b_eff = (han_b - han_w.sum(axis=0)).reshape(1, D).astype(ml_dtypes.bfloat16)
    W16 = W.astype(f16)
    han16 = han_w.astype(f16)

    nc = _get_nc()
    in_maps = [
        {
            "x16": np.ascontiguousarray(x[b]).astype(f16),
            "adj16": adj[b].astype(f16),
            "W16": W16,
            "a_rs": a_rs,
            "han16": han16,
            "b_eff": b_eff,
        }
        for b in range(B)
    ]
    last_err = None
    for attempt in range(3):
        try:
            res = run_bass_kernel_spmd(nc, in_maps, core_ids=list(range(B)),
                                       trace=trace)
            out = np.stack([np.asarray(r["out"]) for r in res.results], axis=0)
            return out, res
        except Exception as e:  # transient NRT/axon execute failures
            last_err = e
            import time as _time
            _time.sleep(3.0 + 5.0 * attempt)
    raise last_err


def kernel(**inputs) -> np.ndarray:
    out, _ = run(inputs, trace=False)
    return out
